# revision 13
# baseline (speedup 1.0000x reference)
"""Complex 2x2 nearest-neighbor upsampling on 8 Trainium2 NeuronCores.

out[b, i, j, c] = complex(x_re, x_im)[b, i//2, j//2, c]

Full shapes: x_re/x_im f32 [16, 128, 128, 64] -> out complex64 [16, 256, 256, 64].

Strategy (pure data movement, memory-bound):
  - Batch-parallel: 2 images per core (16 / 8).
  - SBUF layout: partition p = input row h, free dim = a chunk of WC input
    pixels x 64 channels. Input DMAs are [128 x WC*256B-contiguous] reads.
  - DVE builds the fully interleaved, width-duplicated output chunk in SBUF:
    free dim (w, dup_w, c, re/im). 4 strided copies per chunk.
  - Each SBUF output chunk is DMA'd to HBM twice (duplicate output rows
    2h and 2h+1), each DMA [128 partitions x WC*1KB contiguous].
  - Raw bass pipeline spread across DMA queues: SWDGE (gpsimd) issues the
    loads, the two HWDGE queues (SP + ACT) each issue one of the two
    duplicate-row stores. Standalone wait instructions only (the direct2d
    DMA encoding allows a single inline wait).
  - Host views the f32 [.., 64, 2] output as complex64 and concatenates.
"""

import sys
from contextlib import ExitStack

import numpy as np

for _p in ("/opt/trn_rl_repo", "/root/.axon_site/_ro/trn_rl_repo"):
    if _p not in sys.path:
        sys.path.append(_p)

import concourse.bass as bass
import concourse.mybir as mybir
from concourse.bass_utils import run_bass_kernel_spmd

N_CORES = 8
B_FULL = 16
B = B_FULL // N_CORES  # images per core
H = 128
W = 128
C = 64
HO = 2 * H
WO = 2 * W

_cached = None


def build_nc(
    reps: int = 1,
    wc: int = 32,
    in_bufs: int = 4,
    out_bufs: int = 3,
    store_split: str = "2way",
):
    nchunk = W // wc
    niter = B * nchunk

    nc = bass.Bass()
    x_re = nc.dram_tensor("x_re", [B, H, W, C], mybir.dt.float32, kind="ExternalInput")
    x_im = nc.dram_tensor("x_im", [B, H, W, C], mybir.dt.float32, kind="ExternalInput")
    # f32 view of the complex64 output: last dim interleaves (c, re/im)
    out = nc.dram_tensor(
        "out", [B, HO, WO, 2 * C], mybir.dt.float32, kind="ExternalOutput"
    )

    f32 = mybir.dt.float32

    def in_src(x, i):
        b, w0 = divmod(i % niter, nchunk)
        w0 *= wc
        return x[b, :, w0 : w0 + wc, :].rearrange("h w c -> h (w c)")

    def out_dst(i, dh):
        b, w0 = divmod(i % niter, nchunk)
        w0 *= wc
        ob = out[b].rearrange("(h two) wo cr -> h two (wo cr)", two=2)
        return ob[:, dh, 2 * w0 * 2 * C : 2 * (w0 + wc) * 2 * C]

    # which engine issues the store for (iter, dh)?  "2way": SP gets dh=0,
    # ACT gets dh=1.  "3way": rotate (SP, ACT, Pool) over the 2*niter stores
    # so the load queue (Pool/SWDGE) carries a share of the stores too.
    def store_engine(i, dh):
        if store_split == "2way":
            return ("sync", "scalar")[dh]
        return ("sync", "scalar", "gpsimd")[(2 * i + dh) % 3]

    with (
        ExitStack() as stack,
        nc.semaphore() as s_copy,
        nc.Block() as block,
    ):
        s_load = [
            stack.enter_context(nc.semaphore(f"s_load{j}")) for j in range(in_bufs)
        ]
        s_out = [
            stack.enter_context(nc.semaphore(f"s_out{j}")) for j in range(out_bufs)
        ]
        s_outg = [
            stack.enter_context(nc.semaphore(f"s_outg{j}")) for j in range(out_bufs)
        ]
        t_re = [
            stack.enter_context(nc.sbuf_tensor(f"t_re{j}", [H, wc * C], f32))
            for j in range(in_bufs)
        ]
        t_im = [
            stack.enter_context(nc.sbuf_tensor(f"t_im{j}", [H, wc * C], f32))
            for j in range(in_bufs)
        ]
        t_out = [
            stack.enter_context(nc.sbuf_tensor(f"t_out{j}", [H, wc * 2 * C * 2], f32))
            for j in range(out_bufs)
        ]

        # cumulative per-slot store-completion sem values after each iter,
        # split by HWDGE (SP/ACT share s_out) vs SWDGE (gpsimd, s_outg)
        total_iters = reps * niter
        cum_hw = [0] * total_iters
        cum_g = [0] * total_iters
        run_hw = [0] * out_bufs
        run_g = [0] * out_bufs
        for j in range(total_iters):
            so_ = j % out_bufs
            for dh in range(2):
                if store_engine(j, dh) == "gpsimd":
                    run_g[so_] += 16
                else:
                    run_hw[so_] += 16
            cum_hw[j] = run_hw[so_]
            cum_g[j] = run_g[so_]

        def emit_store(eng, i, dh):
            eng.wait_ge(s_copy, 4 * (i + 1))
            sem = s_outg if store_engine(i, dh) == "gpsimd" else s_out
            eng.dma_start(out=out_dst(i, dh), in_=t_out[i % out_bufs][:, :]).then_inc(
                sem[i % out_bufs], 16
            )

        @block.gpsimd
        def _(gpsimd):
            for i in range(reps * niter):
                s = i % in_bufs
                if i >= in_bufs:
                    # copies of iter i-in_bufs have finished reading this slot
                    gpsimd.wait_ge(s_copy, 4 * (i - in_bufs + 1))
                gpsimd.dma_start(out=t_re[s][:, :], in_=in_src(x_re, i)).then_inc(
                    s_load[s], 16
                )
                gpsimd.dma_start(out=t_im[s][:, :], in_=in_src(x_im, i)).then_inc(
                    s_load[s], 16
                )
                for dh in range(2):
                    if store_engine(i, dh) == "gpsimd":
                        emit_store(gpsimd, i, dh)

        @block.vector
        def _(vector):
            for i in range(reps * niter):
                s = i % in_bufs
                so = i % out_bufs
                vector.wait_ge(s_load[s], 32 * (i // in_bufs + 1))
                if i >= out_bufs:
                    # stores of iter i-out_bufs have finished reading this slot
                    j = i - out_bufs
                    engines_j = {store_engine(j, dh) for dh in range(2)}
                    if engines_j - {"gpsimd"}:
                        vector.wait_ge(s_out[so], cum_hw[j])
                    if "gpsimd" in engines_j:
                        vector.wait_ge(s_outg[so], cum_g[j])
                ov = t_out[so][:, :].rearrange(
                    "p (w dk c ri) -> p w dk c ri", w=wc, dk=2, c=C, ri=2
                )
                ir = t_re[s][:, :].rearrange("p (w c) -> p w c", w=wc)
                ii = t_im[s][:, :].rearrange("p (w c) -> p w c", w=wc)
                vector.tensor_copy(ov[:, :, 0, :, 0], ir).then_inc(s_copy, 1)
                vector.tensor_copy(ov[:, :, 1, :, 0], ir).then_inc(s_copy, 1)
                vector.tensor_copy(ov[:, :, 0, :, 1], ii).then_inc(s_copy, 1)
                vector.tensor_copy(ov[:, :, 1, :, 1], ii).then_inc(s_copy, 1)

        @block.sync
        def _(sync):
            for i in range(reps * niter):
                for dh in range(2):
                    if store_engine(i, dh) == "sync":
                        emit_store(sync, i, dh)

        @block.scalar
        def _(scalar):
            for i in range(reps * niter):
                for dh in range(2):
                    if store_engine(i, dh) == "scalar":
                        emit_store(scalar, i, dh)

    return nc


def kernel(x_re: np.ndarray, x_im: np.ndarray) -> np.ndarray:
    global _cached
    if _cached is None:
        _cached = build_nc()
    nc = _cached

    x_re = np.asarray(x_re, dtype=np.float32)
    x_im = np.asarray(x_im, dtype=np.float32)

    in_maps = [
        {
            "x_re": np.ascontiguousarray(x_re[B * c : B * (c + 1)]),
            "x_im": np.ascontiguousarray(x_im[B * c : B * (c + 1)]),
        }
        for c in range(N_CORES)
    ]
    res = run_bass_kernel_spmd(nc, in_maps, core_ids=list(range(N_CORES)))
    parts = [
        np.ascontiguousarray(r["out"]).view(np.complex64).reshape(B, HO, WO, C)
        for r in res.results
    ]
    return np.concatenate(parts, axis=0)
